# revision 1
# baseline (speedup 1.0000x reference)
"""Cross-attention (ALiBi) Trainium2 kernel, banded + rebalanced.

Sharding: 8 cores = 2 batches x 4 head-slot-groups. Head dealing is by ALiBi
window so every core gets the same banded tile pattern (SPMD-safe):
  core (b, g) slots = [12+g, 8+g, 4+g, g]   (windows ~[full, full, 176, 38])
  pr0 = slots 0,1 -> all 16 j-tiles;  pr1 = slots 2,3 -> banded j-tiles.

Per core: q/k/v projections for its 4 heads, local banded attention with
multiplicative ALiBi (Toeplitz strip), partial output projection (row-sharded
Wo) DMA'd straight from PSUM; host sums partials + bo.

Layouts (per core):
  qT, kT  [128, 2 pr, 2048 n] bf16   slot s: pr=s//2, partitions (s%2)*64..
  v       [128, 16 jt, 65*4] bf16    slot s cols 65s..65s+64, ones col 65s+64
  scoresT [j, i] per head; softmax denom via ones-column in v
  ALiBi multiplicative: p = exp(s/8) * estrip, estrip sliced from
  [128, 3968] per head, u0 = 1920 - 128*jt + 512*ic.
"""

import sys
import numpy as np
import ml_dtypes
from contextlib import ExitStack

if "/opt/trn_rl_repo" not in sys.path:
    sys.path.insert(0, "/opt/trn_rl_repo")

B, N, E, H, D = 2, 2048, 1024, 16, 64
HPC = 4            # heads per core
ES = HPC * D       # 256 e'-columns per core
NCORES = 8
KT = E // 128      # 8 contraction tiles for projections
NT = N // 128      # 16 n/j tiles
NC512 = N // 512   # 4 chunks of 512
USTRIP = 3968

BF16 = ml_dtypes.bfloat16

# banded j-tile ranges for pr1 (slots 2,3; window 176): per ic, [first, last]
PR1_JTS = [(0, 5), (2, 9), (6, 13), (10, 15)]


def _jts(pr, ic):
    if pr == 0:
        return range(NT)
    lo, hi = PR1_JTS[ic]
    return range(lo, hi + 1)


# slot -> head for group g: heads [12+g, 8+g, 4+g, g]
def _slot_heads(g):
    return [12 + g, 8 + g, 4 + g, g]


_cache: dict = {}


def _alibi_slopes():
    return np.array([2.0 ** (-8.0 * (h + 1) / H) for h in range(H)], dtype=np.float64)


def _estrips():
    """[4 groups][4 slots, 128, 3968] bf16: strip[p, u] = exp(-slope*|p+1920-u|)."""
    if "estrips" in _cache:
        return _cache["estrips"]
    slopes = _alibi_slopes()
    au = np.abs(np.arange(128)[:, None] + 1920 - np.arange(USTRIP)[None, :]).astype(np.float64)
    groups = []
    for g in range(4):
        heads = []
        for h in _slot_heads(g):
            heads.append(np.exp(-slopes[h] * au))
        groups.append(np.stack(heads).astype(BF16))
    _cache["estrips"] = groups
    return groups


def _build():
    import concourse.bass as bass
    import concourse.mybir as mybir
    import concourse.tile as tile
    from concourse import bacc

    fp32 = mybir.dt.float32
    bf16 = mybir.dt.bfloat16
    AF = mybir.ActivationFunctionType

    nc = bacc.Bacc("TRN2", target_bir_lowering=False, debug=False)

    qtt = nc.dram_tensor("qtt", [E, N], bf16, kind="ExternalInput").ap()
    kvt = nc.dram_tensor("kvt", [E, N], bf16, kind="ExternalInput").ap()
    # weights pre-arranged host-side to SBUF layout (plain 2D DMAs)
    wq = nc.dram_tensor("wq", [128, KT * ES], bf16, kind="ExternalInput").ap()
    wk = nc.dram_tensor("wk", [128, KT * ES], bf16, kind="ExternalInput").ap()
    wv = nc.dram_tensor("wv", [128, KT * ES], bf16, kind="ExternalInput").ap()
    wo = nc.dram_tensor("wo", [128, 2 * E], bf16, kind="ExternalInput").ap()
    bq = nc.dram_tensor("bq", [1, ES], bf16, kind="ExternalInput").ap()
    bk = nc.dram_tensor("bk", [1, ES], bf16, kind="ExternalInput").ap()
    bv = nc.dram_tensor("bv", [1, ES], bf16, kind="ExternalInput").ap()
    estrip = nc.dram_tensor("estrip", [128, HPC * USTRIP], bf16, kind="ExternalInput").ap()
    out = nc.dram_tensor("out", [N, E], bf16, kind="ExternalOutput").ap()

    with tile.TileContext(nc) as tc, ExitStack() as ctx:
        consts = ctx.enter_context(tc.tile_pool(name="consts", bufs=1))
        big = ctx.enter_context(tc.tile_pool(name="big", bufs=1))
        acts = ctx.enter_context(tc.tile_pool(name="acts", bufs=1))
        ptpool = ctx.enter_context(tc.tile_pool(name="ptpool", bufs=8))
        small = ctx.enter_context(tc.tile_pool(name="small", bufs=2))
        outsb = ctx.enter_context(tc.tile_pool(name="outsb", bufs=3))
        mmps = ctx.enter_context(tc.tile_pool(name="mmps", bufs=2, space="PSUM"))
        sps = ctx.enter_context(tc.tile_pool(name="sps", bufs=2, space="PSUM"))
        ops = ctx.enter_context(tc.tile_pool(name="ops", bufs=2, space="PSUM"))

        # ---- DMA: projection weights first, then inputs (k-tile interleaved),
        # then out-proj weights / biases, estrip LAST (needed only when
        # attention starts) ----
        wq_sb = consts.tile([128, KT, ES], bf16)
        nc.sync.dma_start(wq_sb[:], wq.rearrange("p (t m) -> p t m", t=KT))
        wk_sb = consts.tile([128, KT, ES], bf16)
        nc.sync.dma_start(wk_sb[:], wk.rearrange("p (t m) -> p t m", t=KT))
        wv_sb = consts.tile([128, KT, ES], bf16)
        nc.scalar.dma_start(wv_sb[:], wv.rearrange("p (t m) -> p t m", t=KT))
        bq_sb = consts.tile([1, ES], bf16)
        nc.sync.dma_start(bq_sb[:], bq)
        bk_sb = consts.tile([1, ES], bf16)
        nc.sync.dma_start(bk_sb[:], bk)
        bv_sb = consts.tile([1, ES], bf16)
        nc.scalar.dma_start(bv_sb[:], bv)

        qtt_sb = big.tile([128, KT, N], bf16)
        kvt_sb = big.tile([128, KT, N], bf16)
        for k in range(KT):
            # round-robin the input tiles over the SP and ACT DMA queues
            qq = nc.sync if k % 2 == 0 else nc.scalar
            kq = nc.scalar if k % 2 == 0 else nc.sync
            qq.dma_start(qtt_sb[:, k, :], qtt[k * 128:(k + 1) * 128, :])
            kq.dma_start(kvt_sb[:, k, :], kvt[k * 128:(k + 1) * 128, :])

        wo_sb = consts.tile([128, 2, E], bf16)
        nc.sync.dma_start(wo_sb[:], wo.rearrange("p (t e) -> p t e", t=2))
        es_sb = consts.tile([128, HPC, USTRIP], bf16)
        nc.sync.dma_start(es_sb[:], estrip.rearrange("p (h u) -> p h u", h=HPC))

        ones_bf = consts.tile([1, 512], bf16)
        nc.vector.memset(ones_bf[:], 1.0)

        qT_sb = acts.tile([128, 2, N], bf16)
        kT_sb = acts.tile([128, 2, N], bf16)
        v_sb = acts.tile([128, NT, 65 * HPC], bf16)
        oT_sb = acts.tile([128, 2, N], bf16)

        # ---- q/k projections: out [e'=128 tile t, n chunk c] ----
        for t in range(2):
            for c in range(NC512):
                for (w_sb, b_sb, dst) in ((wq_sb, bq_sb, qT_sb), (wk_sb, bk_sb, kT_sb)):
                    ps = mmps.tile([128, 512], fp32)
                    for k in range(KT):
                        nc.tensor.matmul(
                            ps[:],
                            w_sb[:, k, t * 128:(t + 1) * 128],
                            qtt_sb[:, k, c * 512:(c + 1) * 512] if w_sb is wq_sb
                            else kvt_sb[:, k, c * 512:(c + 1) * 512],
                            start=(k == 0), stop=False,
                        )
                    nc.tensor.matmul(
                        ps[:], b_sb[:, t * 128:(t + 1) * 128], ones_bf[:, 0:512],
                        start=False, stop=True,
                    )
                    nc.scalar.copy(dst[:, t, c * 512:(c + 1) * 512], ps[:])

        # ---- v projection: out [n tile jt, e'] + ones cols ----
        nc.vector.memset(v_sb[:, :, :].rearrange("p t (h c) -> p t h c", c=65)[:, :, :, 64:65], 1.0)
        for jt in range(NT):
            ps = mmps.tile([128, ES], fp32)
            for k in range(KT):
                nc.tensor.matmul(
                    ps[:],
                    kvt_sb[:, k, jt * 128:(jt + 1) * 128],
                    wv_sb[:, k, :],
                    start=(k == 0), stop=False,
                )
            nc.tensor.matmul(
                ps[:], ones_bf[:, 0:128], bv_sb[:], start=False, stop=True,
            )
            nc.scalar.copy(
                v_sb[:, jt, :].rearrange("p (h c) -> p h c", c=65)[:, :, 0:64],
                ps[:].rearrange("p (h c) -> p h c", c=64),
            )

        # ---- attention: i-chunk outer, head-pair inner (banded for pr1) ----
        pending_norm = None
        outproj_q = []

        def emit_outproj_group(nt, ec):
            ps = mmps.tile([128, 512], fp32)
            for t in range(2):
                nc.tensor.matmul(
                    ps[:],
                    oT_sb[:, t, nt * 128:(nt + 1) * 128],
                    wo_sb[:, t, ec * 512:(ec + 1) * 512],
                    start=(t == 0), stop=(t == 1),
                )
            o_sb = outsb.tile([128, 512], bf16)
            if ec == 0:
                nc.scalar.copy(o_sb[:], ps[:])
            else:
                nc.vector.tensor_copy(o_sb[:], ps[:])
            nc.sync.dma_start(
                out[nt * 128:(nt + 1) * 128, ec * 512:(ec + 1) * 512], o_sb[:])

        for ic in range(NC512):
            isl = slice(ic * 512, (ic + 1) * 512)
            for pr in range(2):
                jts = list(_jts(pr, ic))
                o_pair = []
                for h2 in range(2):
                    o_ps = ops.tile([65, 512], fp32, name=f"o_ps_{h2}", tag="o_ps")
                    o_pair.append(o_ps)

                def emit_norm(pr0, o_un, recip, isl0):
                    # recip already issued at pair end; broadcast + multiply
                    rb = small.tile([64, 2, 512], fp32, name="rb", tag="rb")
                    nc.gpsimd.partition_broadcast(rb[:], recip[:])
                    for h2 in range(2):
                        hp = h2 * 64
                        nc.vector.tensor_mul(
                            oT_sb[hp:hp + 64, pr0, isl0], o_un[0:64, h2, :],
                            rb[:, h2, :])

                def emit_ot(jt, pt2, first, last):
                    for h2 in range(2):
                        s = 2 * pr + h2
                        nc.tensor.matmul(
                            o_pair[h2][:],
                            v_sb[:, jt, s * 65:s * 65 + 65],
                            pt2[:, h2, :],
                            start=first, stop=last,
                        )

                prev = None
                for idx, jt in enumerate(jts):
                    s2 = sps.tile([128, 2, 512], fp32, tag="s_ps", name="s2")
                    for h2 in range(2):
                        hp = h2 * 64
                        nc.tensor.matmul(
                            s2[:, h2, :],
                            kT_sb[hp:hp + 64, pr, jt * 128:(jt + 1) * 128],
                            qT_sb[hp:hp + 64, pr, isl],
                            start=True, stop=True,
                        )
                    pt2 = ptpool.tile([128, 2, 512], bf16, tag="pt", name="pt2")
                    nc.scalar.activation(pt2[:], s2[:], AF.Exp, scale=0.125)
                    u0 = 1920 - 128 * jt + 512 * ic
                    nc.vector.tensor_mul(
                        pt2[:], pt2[:],
                        es_sb[:, 2 * pr:2 * pr + 2, u0:u0 + 512])
                    if idx == 1 and pending_norm is not None:
                        emit_norm(*pending_norm)
                        pending_norm = None
                    if outproj_q and (
                        (pr == 0 and idx in (4, 7, 10, 13))
                        or (pr == 1 and 1 <= idx <= 4)
                    ):
                        emit_outproj_group(*outproj_q.pop(0))
                    if prev is not None:
                        emit_ot(*prev)
                    prev = (jt, pt2, jt == jts[0], jt == jts[-1])
                emit_ot(*prev)
                # free PSUM now; reciprocals issue immediately (ahead of the
                # next pair's DVE muls); broadcast+normalize deferred
                o_un = small.tile([65, 2, 512], fp32, tag="o_un", name="o_un")
                for h2 in range(2):
                    nc.vector.tensor_copy(o_un[:, h2, :], o_pair[h2][:])
                recip = small.tile([1, 2, 512], fp32, name="recip", tag="recip")
                for h2 in range(2):
                    nc.vector.reciprocal(recip[:, h2, :], o_un[64:65, h2, :])
                pending_norm = (pr, o_un, recip, isl)
            # enqueue this i-chunk's output projection, spread one group per
            # j-tile across the next i-chunk's loops
            outproj_q.extend(
                (nt, ec) for nt in range(4 * ic, 4 * ic + 4) for ec in range(2))
        if pending_norm is not None:
            emit_norm(*pending_norm)
            pending_norm = None
        while outproj_q:
            emit_outproj_group(*outproj_q.pop(0))

    nc.compile()
    return nc


def _get_nc():
    if "nc" not in _cache:
        _cache["nc"] = _build()
    return _cache["nc"]


def _warr(w):
    """[E, ES] -> [128, KT*ES] sbuf layout: row p = concat_k w[k*128+p, :]."""
    return np.ascontiguousarray(
        w.reshape(KT, 128, ES).transpose(1, 0, 2).reshape(128, KT * ES)
    ).astype(BF16)


def _in_maps(query, kv, Wq, bq, Wkv, bkv, Wo, bo):
    strips = _estrips()
    qT = [np.ascontiguousarray(query[b].T).astype(BF16) for b in range(B)]
    kvT = [np.ascontiguousarray(kv[b].T).astype(BF16) for b in range(B)]
    maps = []
    for c in range(NCORES):
        b, g = c // 4, c % 4
        heads = _slot_heads(g)
        cols = np.concatenate([np.arange(h * D, (h + 1) * D) for h in heads])
        wo_arr = np.ascontiguousarray(
            Wo[cols, :].reshape(2, 128, E).transpose(1, 0, 2).reshape(128, 2 * E)
        ).astype(BF16)
        es_arr = np.ascontiguousarray(
            strips[g].transpose(1, 0, 2).reshape(128, HPC * USTRIP))
        maps.append({
            "qtt": qT[b],
            "kvt": kvT[b],
            "wq": _warr(Wq[:, cols]),
            "wk": _warr(Wkv[:, :E][:, cols]),
            "wv": _warr(Wkv[:, E:][:, cols]),
            "wo": wo_arr,
            "bq": np.ascontiguousarray(bq[cols]).reshape(1, ES).astype(BF16),
            "bk": np.ascontiguousarray(bkv[:E][cols]).reshape(1, ES).astype(BF16),
            "bv": np.ascontiguousarray(bkv[E:][cols]).reshape(1, ES).astype(BF16),
            "estrip": es_arr,
        })
    return maps


def kernel(query, kv, Wq, bq, Wkv, bkv, Wo, bo, _collect=None):
    from concourse import bass_utils

    query = np.asarray(query, dtype=np.float32)
    kv = np.asarray(kv, dtype=np.float32)
    nc = _get_nc()
    maps = _in_maps(query, kv, np.asarray(Wq), np.asarray(bq), np.asarray(Wkv),
                    np.asarray(bkv), np.asarray(Wo), np.asarray(bo))
    res = bass_utils.run_bass_kernel_spmd(
        nc, maps, core_ids=list(range(NCORES)),
        **(_collect or {}),
    )
    if _collect is not None:
        _cache["last_results"] = res
    outp = np.zeros((B, N, E), dtype=np.float32)
    for c in range(NCORES):
        outp[c // 4] += res.results[c]["out"].astype(np.float32)
    outp += np.asarray(bo, dtype=np.float32)
    return outp

